# revision 1
# baseline (speedup 1.0000x reference)
"""Distributed Trainium2 Bass kernel for fused LayerNorm + causal multi-head
attention + output projection (B=2, T=2048, DIM=1024, H=16, D=64) on 8 cores.

Structure (v3):
  - LayerNorm: token-parallel (512 tokens/core), stats via bn_stats on DVE.
  - QKV: each core computes ALL heads' q/k/v for its OWN 512 tokens, then
    redistributes via two bf16 AllToAlls (K first - it is staged earliest and
    absorbs the ~67us collectives-firmware cold start - then V+Q merged) so
    attention is head-parallel (2 heads x 2 batches per core).
  - Attention: causal-trimmed score/exp/PV tiles, exp over both heads in one
    ACT instruction, diagonal-only masking, late normalization (PV results
    copied out of PSUM immediately; 1/sum applied from SBUF).
  - Keep-warm dummy matmuls hold the PE p-state up across collective waits
    and ACT-bound attention bubbles (tensor clock drops ~2x otherwise).
  - Output: bf16 AllToAll back to token-parallel, ck-outer projection with
    per-accumulator bias/copy/DMA pipelining.

Compute dtype: bf16 matmuls with fp32 PSUM accumulation.
LN affine params and the 1/sqrt(D) score scale are folded into the QKV
weights on the host.
"""
import os
import sys
import types
import numpy as np
import ml_dtypes

# ---------------------------------------------------------------- constants
B, T, DIM, D = 2, 2048, 1024, 64
H = DIM // D            # 16 heads
NC = 8                  # cores
TOK = B * T             # 4096 tokens
TPC = TOK // NC         # 512 tokens per core
KT8 = DIM // 128        # 8 contraction tiles
EPS = 1e-5

TRACE = bool(int(os.environ.get("BASS_KERNEL_TRACE", "0")))
DUM_LN = int(os.environ.get("DUM_LN", "100"))
DUM_KV = int(os.environ.get("DUM_KV", "130"))
DUM_Q = int(os.environ.get("DUM_Q", "44"))
DUM_AT = int(os.environ.get("DUM_AT", "60"))
DUMF_W = int(os.environ.get("DUMF_W", "256"))
BN_LN = bool(int(os.environ.get("BN_LN", "1")))
MERGE_VTR = bool(int(os.environ.get("MERGE_VTR", "1")))

BF16_NP = ml_dtypes.bfloat16


def _ensure_ntff_hook():
    """The agent image lacks antenv.axon_hooks; recreate it so trace=True works."""
    if "antenv.axon_hooks" not in sys.modules:
        mod = types.ModuleType("antenv.axon_hooks")
        mod._hook = None
        def set_axon_ntff_profile_hook(h):
            mod._hook = h
        def get_axon_ntff_profile_hook():
            return mod._hook
        mod.set_axon_ntff_profile_hook = set_axon_ntff_profile_hook
        mod.get_axon_ntff_profile_hook = get_axon_ntff_profile_hook
        sys.modules["antenv.axon_hooks"] = mod
    m = sys.modules["antenv.axon_hooks"]
    if m.get_axon_ntff_profile_hook() is None:
        try:
            from trn_agent_boot.trn_boot import _ntff_profile_via_ctypes
            m.set_axon_ntff_profile_hook(
                _ntff_profile_via_ctypes("/opt/axon/libaxon_pjrt.so"))
        except Exception:
            pass


def build_graph():
    import concourse.bass as bass
    import concourse.bacc as bacc
    import concourse.tile as tile
    import concourse.mybir as mybir

    dt = mybir.dt
    F32, BF16 = dt.float32, dt.bfloat16
    AF = mybir.ActivationFunctionType
    ALU = mybir.AluOpType
    RG = [list(range(NC))]

    nc = bacc.Bacc(None, target_bir_lowering=False, debug=False, num_devices=NC)

    # ------------------------------------------------------------ I/O
    x_in = nc.dram_tensor("x_c", [TPC, DIM], F32, kind="ExternalInput")
    wkv_in = nc.dram_tensor("wkv", [128, KT8 * 2048], BF16, kind="ExternalInput")
    wq_in = nc.dram_tensor("wq", [128, KT8 * 1024], BF16, kind="ExternalInput")
    bkv_in = nc.dram_tensor("bkv", [128, 16], F32, kind="ExternalInput")
    bq_in = nc.dram_tensor("bq", [128, 8], F32, kind="ExternalInput")
    pwt_in = nc.dram_tensor("pwt", [128, KT8 * DIM], BF16, kind="ExternalInput")
    pb_in = nc.dram_tensor("pb", [1, DIM], BF16, kind="ExternalInput")
    idn_in = nc.dram_tensor("idn", [128, 128], BF16, kind="ExternalInput")
    ones_in = nc.dram_tensor("ones_r", [1, 128], BF16, kind="ExternalInput")
    emat_in = nc.dram_tensor("emat", [33, 128], BF16, kind="ExternalInput")
    out_dram = nc.dram_tensor("out_c", [TPC, DIM], F32, kind="ExternalOutput")

    with tile.TileContext(nc) as tc:
        with (
            tc.tile_pool(name="persist", bufs=1) as pers,
            tc.tile_pool(name="dram", bufs=1, space="DRAM") as dram,
        ):
            # ---------------- DRAM bounce buffers ----------------
            k_in = dram.tile([NC * 128, TPC], BF16)
            k_out = dram.tile([NC * 128, TPC], BF16)
            vq_in = dram.tile([NC * 256, TPC], BF16)
            vq_out = dram.tile([NC * 256, TPC], BF16)
            a2a_in = dram.tile([NC * 128, TPC], BF16)
            a2a_out = dram.tile([NC * 128, TPC], BF16)

            # idn first: transposes + dummies need it early; it is tiny
            idn_sb = pers.tile([128, 128], BF16)
            nc.sync.dma_start(idn_sb[:], idn_in[:])


            # x tiles FIRST (LN critical path), then QKV weights.
            # (pwt/pb/ones/emat are deferred until after the KV A2A trigger so
            # their descriptors cannot delay the critical staging DMAs.)
            xts = []
            with tc.tile_pool(name="ln_x", bufs=1) as lnx:
                for t in range(4):
                    xt = lnx.tile([128, DIM], F32, tag=f"xt{t}", name=f"xt{t}")
                    for hh in range(2):
                        nc.sync.dma_start(
                            xt[:, 512 * hh:512 * (hh + 1)],
                            x_in[128 * t:128 * (t + 1), 512 * hh:512 * (hh + 1)])
                    xts.append(xt)
                wkv_sb = pers.tile([128, KT8 * 2048], BF16)
                for i in range(4):
                    nc.sync.dma_start(wkv_sb[:, 4096 * i:4096 * (i + 1)],
                                      wkv_in[:, 4096 * i:4096 * (i + 1)])
                wq_sb = pers.tile([128, KT8 * 1024], BF16)
                for i in range(2):
                    nc.sync.dma_start(wq_sb[:, 4096 * i:4096 * (i + 1)],
                                      wq_in[:, 4096 * i:4096 * (i + 1)])
                bkv_sb = pers.tile([128, 16], F32)
                nc.sync.dma_start(bkv_sb[:], bkv_in[:])
                bq_sb = pers.tile([128, 8], F32)
                nc.sync.dma_start(bq_sb[:], bq_in[:])

                # ============= P1: LayerNorm (token slice, natural) ========
                xn_sb = pers.tile([128, 4 * DIM], BF16)
                with tc.tile_pool(name="ln", bufs=4) as lnp:
                    if DUM_LN:
                        with tc.tile_pool(name="ps_dln", bufs=1,
                                          space="PSUM") as psdl:
                            dln = psdl.tile([128, 128], BF16, tag="dln")
                            for i in range(DUM_LN):
                                nc.tensor.transpose(dln[:], idn_sb[:], idn_sb[:])
                    for t in range(4):
                        xt = xts[t]
                        stats = lnp.tile([128, 12], F32, tag="stats")
                        nc.vector.bn_stats(stats[:, 0:6], xt[:, 0:512])
                        nc.vector.bn_stats(stats[:, 6:12], xt[:, 512:1024])
                        mv = lnp.tile([128, 2], F32, tag="mv")
                        nc.vector.bn_aggr(mv[:], stats[:])
                        vareps = lnp.tile([128, 1], F32, tag="vareps")
                        nc.vector.tensor_scalar(vareps[:], mv[:, 1:2], 1.0, EPS,
                                                op0=ALU.mult, op1=ALU.add)
                        nmu = lnp.tile([128, 1], F32, tag="nmu")
                        nc.vector.tensor_scalar_mul(nmu[:], mv[:, 0:1], -1.0)
                        std = lnp.tile([128, 1], F32, tag="std")
                        nc.scalar.activation(std[:], vareps[:], AF.Sqrt)
                        rstd = lnp.tile([128, 1], F32, tag="rstd")
                        nc.vector.reciprocal(rstd[:], std[:])
                        nmr = lnp.tile([128, 1], F32, tag="nmr")
                        nc.vector.scalar_tensor_tensor(
                            nmr[:], nmu[:], 1.0, rstd[:],
                            op0=ALU.mult, op1=ALU.mult)
                        nc.scalar.activation(
                            xn_sb[:, DIM * t:DIM * (t + 1)], xt[:],
                            AF.Identity, bias=nmr[:], scale=rstd[:])

            # ================= P2: transpose xn -> xnT =====================
            xnT_sb = pers.tile([128, KT8 * TPC], BF16)  # [dim-tile part, k*512+t]
            with tc.tile_pool(name="ps_tr", bufs=6, space="PSUM") as pstr:
                for t in range(4):
                    for k in range(KT8):
                        trp = pstr.tile([128, 128], BF16, tag="tr")
                        nc.tensor.transpose(
                            trp[:], xn_sb[:, DIM * t + 128 * k: DIM * t + 128 * (k + 1)],
                            idn_sb[:])
                        nc.vector.tensor_copy(
                            xnT_sb[:, TPC * k + 128 * t: TPC * k + 128 * (t + 1)],
                            trp[:])

            # ================= P3/P4: QKV on own tokens + A2As =============
            wkv3 = wkv_sb[:].rearrange("p (k r) -> p k r", r=2048)
            wq3 = wq_sb[:].rearrange("p (k r) -> p k r", r=1024)
            with (
                tc.tile_pool(name="ps_qkv", bufs=3, space="PSUM") as psq,
                tc.tile_pool(name="ps_dum", bufs=1, space="PSUM") as psd,
                tc.tile_pool(name="stg", bufs=4) as stg,
            ):
                dum = psd.tile([128, 512], F32, tag="dum")

                def emit_dummies(n, w=512):
                    for i in range(n):
                        nc.tensor.matmul(dum[:, 0:w], idn_sb[:],
                                         xn_sb[:, 512 * (i % 8):512 * (i % 8) + w],
                                         start=True, stop=True)

                def emit_group(w3, rows0, bias_sb, gi, dst, drow):
                    psg = psq.tile([128, TPC], F32, tag="qg")
                    for k in range(KT8):
                        nc.tensor.matmul(
                            psg[:], w3[:, k, rows0:rows0 + 128],
                            xnT_sb[:, TPC * k:TPC * (k + 1)],
                            start=(k == 0), stop=(k == KT8 - 1))
                    st = stg.tile([128, TPC], BF16, tag="st")
                    nc.vector.tensor_scalar(
                        st[:], psg[:], bias_sb[:, gi:gi + 1], None, op0=ALU.add)
                    nc.sync.dma_start(dst[drow:drow + 128, :], st[:])

                for g in range(0, 16, 2):   # K groups (dest c = g//2)
                    emit_group(wkv3, 128 * g, bkv_sb, g, k_in, 64 * g)
                nc.gpsimd.collective_compute(
                    "AllToAll", ALU.bypass, replica_groups=RG,
                    ins=[k_in[:].opt()], outs=[k_out[:].opt()],
                )
                for g in range(1, 16, 2):   # V groups (dest c = g//2)
                    emit_group(wkv3, 128 * g, bkv_sb, g, vq_in, 128 * (g - 1))

                # deferred weight loads (needed only from attention onwards)
                pwt_sb = pers.tile([128, KT8 * DIM], BF16)
                for i in range(2):
                    nc.sync.dma_start(pwt_sb[:, 4096 * i:4096 * (i + 1)],
                                      pwt_in[:, 4096 * i:4096 * (i + 1)])
                pb_sb = pers.tile([1, DIM], BF16)
                nc.sync.dma_start(pb_sb[:], pb_in[:])
                ones_sb = pers.tile([1, 128], BF16)
                nc.sync.dma_start(ones_sb[:], ones_in[:])
                emat_sb = pers.tile([33, 128], BF16)
                nc.sync.dma_start(emat_sb[:], emat_in[:])

                sums_col = pers.tile([33, 512], F32)
                nc.vector.memset(sums_col[:], 1.0)

                for g in range(8):    # Q groups: dest c = g
                    emit_group(wq3, 128 * g, bq_sb, g, vq_in, 256 * g + 128)
                nc.gpsimd.collective_compute(
                    "AllToAll", ALU.bypass, replica_groups=RG,
                    ins=[vq_in[:].opt()], outs=[vq_out[:].opt()],
                )
                emit_dummies(DUM_KV)

            # ================= P5: unpack + V transposes ===================
            kT_sb = pers.tile([128, TOK], BF16)
            vT_sb = pers.tile([128, TOK], BF16)
            qT_sb = pers.tile([128, TOK], BF16)
            for r in range(NC):
                nc.sync.dma_start(vT_sb[:, TPC * r:TPC * (r + 1)],
                                  vq_out[256 * r:256 * r + 128, :])
            for r in range(NC):
                nc.sync.dma_start(kT_sb[:, TPC * r:TPC * (r + 1)],
                                  k_out[128 * r:128 * (r + 1), :])
            for r in range(NC):
                nc.sync.dma_start(qT_sb[:, TPC * r:TPC * (r + 1)],
                                  vq_out[256 * r + 128:256 * (r + 1), :])

            vnat = []
            for b in range(B):
                vb = pers.tile([128, 16 * 130], BF16, name=f"vnat{b}")
                nc.vector.memset(
                    vb[:].rearrange("p (j a w) -> p j a w", a=2, w=65)[:, :, :, 64:65], 1.0)
                vnat.append(vb)
            attnT = pers.tile([128, TOK], BF16)

            with tc.tile_pool(name="ps_vtr", bufs=4, space="PSUM") as psv:
                for b in range(B):
                    for j in range(16):
                        vtr = psv.tile([128, 128], BF16, tag="vtr")
                        nc.tensor.transpose(
                            vtr[:],
                            vT_sb[:, b * T + 128 * j: b * T + 128 * (j + 1)],
                            idn_sb[:])
                        if MERGE_VTR:
                            nc.vector.tensor_copy(
                                vnat[b][:, 130 * j:130 * (j + 1)]
                                .rearrange("p (a w) -> p a w", w=65)[:, :, 0:64],
                                vtr[:].rearrange("p (a w) -> p a w", w=64))
                        else:
                            nc.vector.tensor_copy(
                                vnat[b][:, 130 * j: 130 * j + 64], vtr[:, 0:64])
                            nc.vector.tensor_copy(
                                vnat[b][:, 130 * j + 65: 130 * j + 129], vtr[:, 64:128])
                with tc.tile_pool(name="ps_dum2", bufs=1, space="PSUM") as psd2:
                    dum2 = psd2.tile([128, 512], F32, tag="dum2")
                    for i in range(DUM_Q):
                        nc.tensor.matmul(dum2[:], idn_sb[:],
                                         xn_sb[:, 512 * (i % 8):512 * (i % 8) + 512],
                                         start=True, stop=True)

            # ================= P6: attention ===============================
            with (
                tc.tile_pool(name="pt", bufs=3) as ptp,
                tc.tile_pool(name="un", bufs=4) as unp,
                tc.tile_pool(name="ps_s", bufs=2, space="PSUM") as pss,
                tc.tile_pool(name="ps_pv", bufs=2, space="PSUM") as psp,
                tc.tile_pool(name="ps_bc", bufs=1, space="PSUM") as psb,
                tc.tile_pool(name="ps_df", bufs=1, space="PSUM") as psf,
                tc.tile_pool(name="sm", bufs=2) as smp,
            ):
                dumf = psf.tile([128, 512], F32, tag="dumf")

                def emit_attention(b):
                    for qc in range(4):
                        q0 = b * T + 512 * qc
                        pvA = psp.tile([65, 512], F32, tag="pv")
                        pvB = psp.tile([65, 512], F32, tag="pv")
                        nkp = 4 * qc + 4
                        pend = None
                        for kp in range(nkp):
                            k0 = b * T + 128 * kp
                            c0 = max(0, 128 * (kp - 4 * qc))
                            sAB = pss.tile([128, 1024], F32, tag="s")
                            nc.tensor.matmul(sAB[:, c0:512],
                                             kT_sb[0:64, k0:k0 + 128],
                                             qT_sb[0:64, q0 + c0:q0 + 512],
                                             start=True, stop=True)
                            nc.tensor.matmul(sAB[:, 512 + c0:1024],
                                             kT_sb[64:128, k0:k0 + 128],
                                             qT_sb[64:128, q0 + c0:q0 + 512],
                                             start=True, stop=True)
                            if pend is not None:
                                pkp, pp, pc0 = pend
                                nc.tensor.matmul(pvA[:, pc0:512],
                                                 vnat[b][:, 130 * pkp:130 * pkp + 65],
                                                 pp[:, pc0:512],
                                                 start=(pkp == 0), stop=False)
                                nc.tensor.matmul(pvB[:, pc0:512],
                                                 vnat[b][:, 130 * pkp + 65:130 * pkp + 130],
                                                 pp[:, 512 + pc0:1024],
                                                 start=(pkp == 0), stop=False)
                            elif DUMF_W:
                                for _ in range(2):
                                    nc.tensor.matmul(dumf[:, 0:DUMF_W], idn_sb[:],
                                                     xn_sb[:, 0:DUMF_W],
                                                     start=True, stop=True)
                            pAB = ptp.tile([128, 1024], BF16, tag="pA")
                            s3 = sAB[:].rearrange("p (j c) -> p j c", j=2)
                            p3 = pAB[:].rearrange("p (j c) -> p j c", j=2)
                            nc.scalar.activation(p3[:, :, c0:512], s3[:, :, c0:512],
                                                 AF.Exp)
                            if kp >= 4 * qc:
                                nc.gpsimd.affine_select(
                                    p3[:, :, c0:c0 + 128], p3[:, :, c0:c0 + 128],
                                    pattern=[[0, 2], [1, 128]],
                                    compare_op=ALU.is_ge, fill=0.0,
                                    base=0, channel_multiplier=-1)
                            if DUMF_W and kp + 1 < nkp:
                                nc.tensor.matmul(dumf[:, 0:DUMF_W], idn_sb[:],
                                                 xn_sb[:, 0:DUMF_W],
                                                 start=True, stop=True)
                            pend = (kp, pAB, c0)
                        pkp, pp, pc0 = pend
                        nc.tensor.matmul(pvA[:, pc0:512],
                                         vnat[b][:, 130 * pkp:130 * pkp + 65],
                                         pp[:, pc0:512],
                                         start=(pkp == 0), stop=True)
                        nc.tensor.matmul(pvB[:, pc0:512],
                                         vnat[b][:, 130 * pkp + 65:130 * pkp + 130],
                                         pp[:, 512 + pc0:1024],
                                         start=(pkp == 0), stop=True)
                        # late normalization: copy PV out of PSUM immediately
                        u = unp.tile([128, 512], BF16, tag="u")
                        nc.vector.tensor_copy(sums_col[0:1, :], pvA[64:65, :])
                        nc.vector.tensor_copy(sums_col[32:33, :], pvB[64:65, :])
                        nc.vector.tensor_copy(u[0:64, :], pvA[0:64, :])
                        nc.vector.tensor_copy(u[64:128, :], pvB[0:64, :])
                        rec = smp.tile([33, 512], F32, tag="rec")
                        nc.vector.reciprocal_approx_fast(rec[:], sums_col[:])
                        recb = smp.tile([33, 512], BF16, tag="recb")
                        nc.vector.tensor_copy(recb[:], rec[:])
                        bc2 = psb.tile([128, 512], F32, tag="bc")
                        nc.tensor.matmul(bc2[:], emat_sb[:], recb[:],
                                         start=True, stop=True)
                        bc2s = smp.tile([128, 512], BF16, tag="bc2s")
                        nc.vector.tensor_copy(bc2s[:], bc2[:])
                        nc.vector.tensor_tensor(
                            attnT[:, q0:q0 + 512], u[:],
                            bc2s[:], op=ALU.mult)
                        nc.sync.dma_start(
                            a2a_in[128 * (4 * b + qc):128 * (4 * b + qc) + 128, :],
                            attnT[:, q0:q0 + 512])

                emit_attention(0)
                emit_attention(1)
                for i in range(12):
                    nc.tensor.matmul(dumf[:], idn_sb[:],
                                     xn_sb[:, 512 * (i % 8):512 * (i % 8) + 512],
                                     start=True, stop=True)

            # ================= P7: AllToAll attention outputs ==============
            nc.gpsimd.collective_compute(
                "AllToAll", ALU.bypass, replica_groups=RG,
                ins=[a2a_in[:].opt()], outs=[a2a_out[:].opt()],
            )

            # ================= P8: output projection (token slice) =========
            pwt3 = pwt_sb[:].rearrange("p (k o) -> p k o", o=DIM)
            with (
                tc.tile_pool(name="projx", bufs=1) as pxp,
                tc.tile_pool(name="ps_o", bufs=1, space="PSUM") as pso,
                tc.tile_pool(name="outp", bufs=1) as outp,
            ):
                accs = [pso.tile([128, 512], F32, tag=f"po{i}", name=f"acc{i}")
                        for i in range(8)]
                ots = [outp.tile([128, DIM], F32, tag=f"ot{t}", name=f"ot{t}")
                       for t in range(4)]
                for i in range(DUM_AT):
                    nc.tensor.matmul(accs[i % 8][:], idn_sb[:],
                                     xn_sb[:, 512 * (i % 8):512 * (i % 8) + 512],
                                     start=True, stop=True)
                aT = []
                for ck in range(KT8):
                    ak = pxp.tile([128, TPC], BF16, tag=f"aT{ck}")
                    nc.sync.dma_start(ak[:],
                                      a2a_out[128 * ck:128 * (ck + 1), :])
                    aT.append(ak)
                for ck in range(KT8):
                    for tt in range(4):
                        for half in range(2):
                            i = 2 * tt + half
                            nc.tensor.matmul(
                                accs[i][:],
                                aT[ck][:, 128 * tt:128 * (tt + 1)],
                                pwt3[:, ck, 512 * half:512 * (half + 1)],
                                start=(ck == 0), stop=False)
                            if ck == KT8 - 1:
                                nc.tensor.matmul(
                                    accs[i][:], ones_sb[0:1, :],
                                    pb_sb[:, 512 * half:512 * (half + 1)],
                                    start=False, stop=True)
                                nc.vector.tensor_copy(
                                    ots[tt][:, 512 * half:512 * (half + 1)],
                                    accs[i][:])
                                if half == 1:
                                    nc.sync.dma_start(
                                        out_dram[128 * tt:128 * (tt + 1), :],
                                        ots[tt][:])

    nc.compile()
    return nc


def host_prep(inputs):
    x = np.asarray(inputs["x"], np.float32).reshape(TOK, DIM)
    ln_w = np.asarray(inputs["ln_w"], np.float32)
    ln_b = np.asarray(inputs["ln_b"], np.float32)
    qkv_w = np.asarray(inputs["qkv_w"], np.float32)
    qkv_b = np.asarray(inputs["qkv_b"], np.float32)
    proj_w = np.asarray(inputs["proj_w"], np.float32)
    proj_b = np.asarray(inputs["proj_b"], np.float32)

    # fold LN affine into qkv weights; fold 1/sqrt(D) into Q rows
    Wp = qkv_w * ln_w[None, :]
    bp = qkv_b + qkv_w @ ln_b
    Wp[0:DIM] *= D ** -0.5
    bp[0:DIM] *= D ** -0.5

    # KV row order: for dest c: K rows (heads 2c,2c+1), V rows (heads 2c,2c+1)
    kv_rows = []
    for c in range(NC):
        for blk in (1, 2):    # K, V
            for h in (2 * c, 2 * c + 1):
                kv_rows.extend(range(blk * DIM + h * D, blk * DIM + (h + 1) * D))
    kv_rows = np.array(kv_rows)
    q_rows = []
    for c in range(NC):
        for h in (2 * c, 2 * c + 1):
            q_rows.extend(range(h * D, (h + 1) * D))
    q_rows = np.array(q_rows)

    def swz(wt):   # [rows, DIM] -> stationary layout [128, KT8 * rows]
        r = wt.shape[0]
        return np.ascontiguousarray(
            wt.T.reshape(KT8, 128, r).transpose(1, 0, 2).reshape(128, KT8 * r)
        ).astype(BF16_NP)

    wkv = swz(Wp[kv_rows])
    wq = swz(Wp[q_rows])
    bkv = np.ascontiguousarray(bp[kv_rows].reshape(16, 128).T)
    bq = np.ascontiguousarray(bp[q_rows].reshape(8, 128).T)

    idn = np.eye(128, dtype=np.float32).astype(BF16_NP)
    ones_r = np.ones((1, 128), BF16_NP)
    emat = np.zeros((33, 128), np.float32)
    emat[0, 0:64] = 1.0
    emat[32, 64:128] = 1.0
    emat = emat.astype(BF16_NP)
    pwt = swz(proj_w)
    pb = proj_b.reshape(1, DIM).astype(BF16_NP)

    in_maps = []
    for c in range(NC):
        in_maps.append(dict(
            x_c=np.ascontiguousarray(x[TPC * c:TPC * (c + 1)]),
            wkv=wkv, wq=wq, bkv=bkv, bq=bq,
            pwt=pwt, pb=pb, idn=idn, ones_r=ones_r, emat=emat,
        ))
    return in_maps


_CACHED = {}


def kernel(**inputs) -> np.ndarray:
    _ensure_ntff_hook()
    from concourse import bass_utils
    if TRACE:
        bass_utils.upload_artifacts = lambda tmpdir: "/tmp/noupload"

    if "nc" not in _CACHED:
        _CACHED["nc"] = build_graph()
    nc = _CACHED["nc"]

    in_maps = host_prep(inputs)
    res = bass_utils.run_bass_kernel_spmd(
        nc, in_maps, core_ids=list(range(NC)), trace=TRACE,
        trace_cores=list(range(NC)) if TRACE else None)
    _CACHED["last_result"] = res
    out = np.concatenate([res.results[c]["out_c"] for c in range(NC)], axis=0)
    return out.reshape(B, T, DIM).astype(np.float32)

